# revision 61
# baseline (speedup 1.0000x reference)
"""Multi-head causal self-attention with RoPE on 8 Trainium2 NeuronCores.

Problem: x:(4,2048,1024) f32, 16 heads, d_k=64, causal, RoPE theta=1e4,
out = softmax(rope(q) rope(k)^T / 8, causal) v, then out-proj.

Sharding: core c handles batch c//2 and heads 8*(c%2) .. 8*(c%2)+8.
Each core computes QKV for its 8 heads (row-sliced weights), causal
attention, and a partial out-projection y_part = attnout_slice @ WoT_slice.
Host sums the two partials per batch.

v2 design (vs v1): all matmul operands bf16 (PSUM accum stays f32), x is
transposed on the host (no PE transposes), attention output stays in SBUF
(no DRAM staging round-trip), the softmax denominator comes from a
64-column ones block in the V stationary operand (rows 64:128 of the
attnV psum hold the denominator on 64 partitions -> plain DVE reciprocal,
no gpsimd partition broadcast), band masks run on DVE in bf16 (one
strided-AP op covers both bands), and the fully-masked leading columns of
the diagonal tiles are never computed (exp over the bounded stale psum is
unread).

Scheduling (per-engine FIFO order == execution order): attention chunks
interleave the two heads pair-by-pair with attnV lagging scores by one
pair; every chunk weaves PE "filler" units (next group's projection
halves, remaining V-projection tiles, or previous chunks' out-projection
pieces) between pairs so the PE stays fed while ACT drains the exp
backlog. Startup: DMAs issue in strict first-use order (transfers
serialize on the DMA engines), dummy matmuls on a zeroed tile warm the
clock gate, and group-0 projection emits at ic-half granularity to match
DMA arrivals; V-proj evacs go through the then-idle ACT engine.

Device layouts (per core):
  xT   [i, s]       - transposed activations (bf16, from host)
  qT,kT[hd, s]      - projections in transposed layout (RoPE'd in place)
  v_sb [s, st,h,128]- cols 0:64 v-dims, cols 64:128 ones (denominator)
  scoresT[sk, sq]   - psum; exp tiles feed attn@V directly as moving operand
  attno[hd, c, s]   - SBUF bf16, feeds out-proj; output written as yT[o, s]

The per-head d_k dims of Wq/Wk are host-permuted (evens then odds) so RoPE
becomes the rotate-half form; scores are invariant to this permutation.
"""

from contextlib import ExitStack

import ml_dtypes
import numpy as np

import concourse.tile as tile
from concourse import bacc, mybir
from concourse.bass_utils import run_bass_kernel_spmd

F32 = mybir.dt.float32
BF16 = mybir.dt.bfloat16
AF = mybir.ActivationFunctionType

D_MODEL = 1024
SEQ = 2048
BATCH = 4
N_HEADS = 16
DK = 64
N_CORES = 8
HPC = 8            # heads per core
HD = HPC * DK      # 512 head-dims per core
P = 128
SC = 512           # seq chunk (matmul moving dim)
NSC = SEQ // SC    # 4
NST = SEQ // P     # 16
NIC = D_MODEL // P # 8
NH4 = HD // P      # 4  (128-dim tiles = 2 heads each)


def build_nc():
    nc = bacc.Bacc("TRN2", target_bir_lowering=False, debug=False)

    xT_d = nc.dram_tensor("xT", [D_MODEL, SEQ], BF16, kind="ExternalInput").ap()
    wq_d = nc.dram_tensor("wqT", [D_MODEL, HD], BF16, kind="ExternalInput").ap()
    wk_d = nc.dram_tensor("wkT", [D_MODEL, HD], BF16, kind="ExternalInput").ap()
    wv_d = nc.dram_tensor("wvT", [D_MODEL, HD], BF16, kind="ExternalInput").ap()
    wo_d = nc.dram_tensor("woT", [HD, D_MODEL], BF16, kind="ExternalInput").ap()
    cos_d = nc.dram_tensor("cosw", [P, SEQ], BF16, kind="ExternalInput").ap()
    sin_d = nc.dram_tensor("sinw", [P, SEQ], BF16, kind="ExternalInput").ap()
    mask_d = nc.dram_tensor("mask", [P, 2 * P], BF16, kind="ExternalInput").ap()
    y_d = nc.dram_tensor("yT", [D_MODEL, SEQ], BF16, kind="ExternalOutput").ap()

    with tile.TileContext(nc) as tc:
        with ExitStack() as ctx:
            _emit(ctx, tc, xT_d, wq_d, wk_d, wv_d, wo_d, cos_d, sin_d,
                  mask_d, y_d)
    nc.compile()
    return nc


def _emit(ctx, tc, xT_d, wq_d, wk_d, wv_d, wo_d, cos_d, sin_d, mask_d, y_d):
    nc = tc.nc

    persist = ctx.enter_context(tc.tile_pool(name="persist", bufs=1))
    cos_sb = persist.tile([P, SEQ], BF16, tag="cos")
    sin_sb = persist.tile([P, SEQ], BF16, tag="sin")
    mask_sb = persist.tile([P, 2, P], BF16, tag="mask")
    xT = persist.tile([P, NIC, SEQ], BF16, tag="xT")
    v_sb = persist.tile([P, NST, HPC, 2 * DK], BF16, tag="v")
    attno = persist.tile([P, NH4, SEQ], BF16, tag="attno")
    wo_sb = persist.tile([P, NH4, D_MODEL], BF16, tag="wo")
    wv_sb = persist.tile([P, NIC, HD], BF16, tag="wv")

    wqk_pre = ctx.enter_context(tc.tile_pool(name="wqk_pre", bufs=1))
    xT_r = xT_d.rearrange("(ic p) s -> p ic s", p=P)
    w_pre = {}
    # DMA order on a queue is the arrival order: interleave weight/x pieces
    # so the first projection matmuls (ic-ordered) start as early as possible
    for name, w_d in (("q", wq_d), ("k", wk_d)):
        w_pre[name] = wqk_pre.tile([P, NIC, P], BF16, tag=f"w{name}0",
                                   name=f"w{name}0")
    w_r = {n: (wq_d if n == "q" else wk_d).rearrange("(ic p) o -> p ic o",
                                                     p=P)
           for n in ("q", "k")}
    # transfers serialize on the DMA engines, so issue strictly in order of
    # first use by the PE pipeline
    nc.sync.dma_start(w_pre["q"][:, 0:4, :], w_r["q"][:, 0:4, 0:P])
    nc.sync.dma_start(xT[:, 0:4, 0:SC], xT_r[:, 0:4, 0:SC])
    nc.sync.dma_start(w_pre["q"][:, 4:8, :], w_r["q"][:, 4:8, 0:P])
    nc.sync.dma_start(xT[:, 4:8, 0:SC], xT_r[:, 4:8, 0:SC])
    nc.sync.dma_start(w_pre["k"], w_r["k"][:, :, 0:P])
    for c in range(1, 4):
        nc.sync.dma_start(xT[:, :, SC * c:SC * (c + 1)],
                          xT_r[:, :, SC * c:SC * (c + 1)])
    nc.sync.dma_start(wv_sb, wv_d.rearrange("(ic p) o -> p ic o", p=P))
    nc.sync.dma_start(cos_sb, cos_d)
    nc.sync.dma_start(sin_sb, sin_d)
    nc.sync.dma_start(mask_sb, mask_d.rearrange("p (a c) -> p a c", a=2))
    nc.sync.dma_start(wo_sb, wo_d.rearrange("(c p) o -> p c o", p=P))

    # ones block for the softmax denominator (attnV psum rows 64:128);
    # on the otherwise-idle Pool engine so DVE stays free for rope evacs
    nc.gpsimd.memset(v_sb[:, :, :, DK:2 * DK], 1.0)

    warm = persist.tile([P, SC], BF16, tag="warm")

    wqk_pool = ctx.enter_context(tc.tile_pool(name="wqk", bufs=3))
    qk_pool = ctx.enter_context(tc.tile_pool(name="qk", bufs=4))
    swp_pool = ctx.enter_context(tc.tile_pool(name="swp", bufs=2))
    exp_pool = ctx.enter_context(tc.tile_pool(name="exp", bufs=4))
    rec_pool = ctx.enter_context(tc.tile_pool(name="rec", bufs=4))
    ys_pool = ctx.enter_context(tc.tile_pool(name="ys", bufs=4))
    ps2_pool = ctx.enter_context(tc.tile_pool(name="ps2", bufs=3, space="PSUM"))
    psatt_pool = ctx.enter_context(tc.tile_pool(name="psatt", bufs=2,
                                                space="PSUM"))

    # dummy matmuls on a zeroed tile keep the PE busy while the first x/W
    # DMAs land, so the HAM clock gate is already released (full rate) when
    # real matmuls start
    nc.vector.memset(warm, 0.0)
    wps = ps2_pool.tile([P, 2 * SC], F32, tag="ps2", name="warm_ps")
    for _ in range(12):
        nc.tensor.matmul(wps[:, 0:SC], lhsT=warm[:, 0:P], rhs=warm,
                         start=True, stop=True)

    def emit_vproj(st_lo, st_hi, evac_act=False):
        for st in range(st_lo, st_hi):
            pst = ps2_pool.tile([P, 2 * SC], F32, tag="ps2", name=f"psv_{st}")
            psv = pst[:, 0:HD]
            for ic in range(NIC):
                nc.tensor.matmul(psv, lhsT=xT[:, ic, P * st:P * (st + 1)],
                                 rhs=wv_sb[:, ic, :],
                                 start=(ic == 0), stop=(ic == NIC - 1))
            src = psv.rearrange("p (h d) -> p h d", h=HPC)
            if evac_act:   # startup: ACT is idle, DVE is busy with rope
                nc.scalar.copy(v_sb[:, st, :, 0:DK], src)
            else:
                nc.vector.tensor_copy(v_sb[:, st, :, 0:DK], src)

    def proj_prepare(h4, name):
        # issue the weight DMA and allocate tiles; MM/rope emission follows
        # later via proj_scp / proj_rope (possibly woven between attn pairs)
        if h4 == 0:
            w_t = w_pre[name]
        else:
            w_d = wq_d if name == "q" else wk_d
            w_t = wqk_pool.tile([P, NIC, P], BF16, tag="wqk")
            nc.sync.dma_start(
                w_t, w_d.rearrange("(ic p) o -> p ic o",
                                   p=P)[:, :, P * h4:P * (h4 + 1)])
        dstT = qk_pool.tile([P, SEQ], BF16, tag=f"{name}T",
                            name=f"{name}T_{h4}")
        swp = swp_pool.tile([P, SEQ], BF16, tag="swp",
                            name=f"swp_{h4}_{name}")
        return dict(h4=h4, name=name, w=w_t, d=dstT, s=swp)

    def proj_half(st, scp, half):
        # 512-wide projection piece; psum evac after the second half
        if half == 0:
            st[f"ps{scp}"] = ps2_pool.tile(
                [P, 2 * SC], F32, tag="ps2",
                name=f"ps2p_{st['h4']}_{st['name']}_{scp}")
        ps2 = st[f"ps{scp}"]
        sc = 2 * scp + half
        for ic in range(NIC):
            nc.tensor.matmul(
                ps2[:, SC * half:SC * (half + 1)],
                lhsT=st["w"][:, ic, :],
                rhs=xT[:, ic, SC * sc:SC * (sc + 1)],
                start=(ic == 0), stop=(ic == NIC - 1))
        if half == 1:
            chunk = slice(2 * SC * scp, 2 * SC * (scp + 1))
            nc.vector.tensor_copy(st["d"][:, chunk], ps2)

    def proj_scp(st, scp):
        proj_half(st, scp, 0)
        proj_half(st, scp, 1)

    def proj_rope(st):
        # partition-swap via full-width DMAs split across two queues (sync
        # HWDGE + gpsimd SWDGE) to halve the descriptor-gen serialization,
        # then dstT = dstT*cos + swp*sin (all DVE, bf16)
        for qi, (o, i) in enumerate(((0, 32), (32, 0), (64, 96), (96, 64))):
            eng = nc.sync if qi % 2 else nc.gpsimd
            eng.dma_start(st["s"][o:o + 32, :], st["d"][i:i + 32, :])
        nc.vector.tensor_mul(st["d"], st["d"], cos_sb)
        nc.vector.tensor_mul(st["s"], st["s"], sin_sb)
        nc.vector.tensor_add(st["d"], st["d"], st["s"])

    def emit_proj(h4, names=("q", "k"), qkT=None):
        if qkT is None:
            qkT = {}
        for name in names:
            st = proj_prepare(h4, name)
            qkT[name] = st["d"]
            proj_scp(st, 0)
            proj_scp(st, 1)
            proj_rope(st)
        return qkT

    def _starts(j, tp):
        # per-m first valid column within each 512 half: fully-masked
        # leading columns of the diagonal tiles are never computed; exp
        # output over the corresponding (bounded) stale psum is unread
        diag = 2 * tp - 4 * j   # -4j..0..2: >=0 on diagonal
        kind = ("full" if diag < 0 else "d01" if diag == 0 else "d23")
        return kind, ((0, P) if kind == "d01" else
                      (2 * P, 3 * P) if kind == "d23" else (0, 0))

    def emit_attn_chunk(h4, qkT_, j, fillers=()):
        # attention for the two heads of group h4, query chunk j; the two
        # heads' chains interleave pair-by-pair so each engine always has
        # the other head's work while semaphores propagate; attnV lags
        # scores by one pair. `fillers` are PE filler closures woven one
        # per pair-step. Diagonal handling:
        #   pair (4j, 4j+1): full exp; band masks at [0:128] (t=4j) and
        #     [640:768] (t=4j+1); t=4j+1 columns [512:640] never computed
        #   pair (4j+2, 4j+3): t=4j+2 restricted to >= 256, t=4j+3 to
        #     >= 384; bands at [256:384] and [896:1024]
        fillers = list(fillers)
        psj = {}

        def do_scores(h4, qkT_, j, tp, hp):
            h = 2 * h4 + hp
            qh = qkT_["q"][64 * hp:64 * hp + 64, :]
            kh = qkT_["k"][64 * hp:64 * hp + 64, :]
            kind, sm = _starts(j, tp)
            ps2 = ps2_pool.tile([P, 2 * SC], F32, tag="ps2",
                                name=f"ps2a_{h}_{j}_{tp}")
            for m in range(2):
                t = 2 * tp + m
                nc.tensor.matmul(
                    ps2[:, SC * m + sm[m]:SC * (m + 1)],
                    lhsT=kh[:, P * t:P * (t + 1)],
                    rhs=qh[:, SC * j + sm[m]:SC * (j + 1)],
                    start=True, stop=True)
            exp2 = exp_pool.tile([P, 2 * SC], BF16, tag="exp",
                                 name=f"exp_{h}_{j}_{tp}")
            if kind == "d23":
                # one ACT op over both 256-wide halves
                nc.scalar.activation(
                    exp2[:].rearrange("p (b c) -> p b c",
                                      b=2)[:, :, 2 * P:SC],
                    ps2[:].rearrange("p (b c) -> p b c",
                                     b=2)[:, :, 2 * P:SC],
                    func=AF.Exp, scale=0.125)
            else:
                nc.scalar.activation(exp2, ps2, func=AF.Exp, scale=0.125)
            if kind != "full":
                # both 128-wide band masks in one strided-AP DVE op: bands
                # sit 640 columns apart in both the d01 and d23 cases
                a0 = 0 if kind == "d01" else 2
                e_r = exp2[:].rearrange("p (a c) -> p a c", c=P)[:, a0::5, :]
                nc.vector.tensor_mul(e_r, e_r, mask_sb)
            return (h4, j, tp, hp, exp2)

        def flush_one():
            h4, j, tp, hp, exp2 = pend.pop(0)
            h = 2 * h4 + hp
            _, sm = _starts(j, tp)
            if tp == 0:
                psj[hp] = psatt_pool.tile([P, SC], F32, tag="psatt",
                                          name=f"psatt_{h}_{j}",
                                          bufs=2)
            for m in range(2):
                t = 2 * tp + m
                nc.tensor.matmul(
                    psj[hp][:, sm[m]:], lhsT=v_sb[:, t, h, :],
                    rhs=exp2[:, SC * m + sm[m]:SC * (m + 1)],
                    start=(t == 0), stop=(t == 4 * j + 3))
            if tp == 2 * j + 1:
                # last pair of this chunk: normalize into SBUF attno
                rec = rec_pool.tile([64, SC], F32, tag="rec",
                                    name=f"rec_{h}_{j}")
                nc.vector.reciprocal(rec, psj[hp][64:128, :])
                nc.vector.tensor_mul(
                    attno[64 * hp:64 * hp + 64, h4, SC * j:SC * (j + 1)],
                    psj[hp][0:64, :], rec)

        pend = []
        for tp in range(2 * j + 2):
            for hp in range(2):
                pend.append(do_scores(h4, qkT_, j, tp, hp))
            while len(pend) > 2:
                flush_one()
            if fillers:
                f = fillers.pop(0)
                if f is not None:
                    f()
        while pend:
            flush_one()
        for f in fillers:
            if f is not None:
                f()

    y_r = y_d.rearrange("(a p) s -> p a s", p=P)

    def outproj_unit(j, otp, evac_act=False):
        psy = ps2_pool.tile([P, 2 * SC], F32, tag="ps2",
                            name=f"psy_{j}_{otp}")
        for half in range(2):
            ot = 2 * otp + half
            for c in range(NH4):
                nc.tensor.matmul(
                    psy[:, SC * half:SC * (half + 1)],
                    lhsT=wo_sb[:, c, P * ot:P * (ot + 1)],
                    rhs=attno[:, c, SC * j:SC * (j + 1)],
                    start=(c == 0), stop=(c == NH4 - 1))
        if evac_act:
            # tail: split the evac per-ot across ACT and DVE (both idle at
            # the end) and spread the y DMAs over three queues so neither
            # the staging tiles nor one DGE serializes the drain
            for half in range(2):
                ys = ys_pool.tile([P, SC], BF16, tag="ys1", bufs=8,
                                  name=f"ys_{j}_{otp}_{half}")
                if half == 0:
                    nc.scalar.copy(ys, psy[:, 0:SC])
                else:
                    nc.vector.tensor_copy(ys, psy[:, SC:2 * SC])
                eng = (nc.sync, nc.scalar, nc.gpsimd)[(2 * otp + half) % 3]
                eng.dma_start(
                    y_r[:, 2 * otp + half, SC * j:SC * (j + 1)], ys)
        else:
            ys = ys_pool.tile([P, 2, SC], BF16, tag="ys",
                              name=f"ys_{j}_{otp}")
            nc.vector.tensor_copy(ys,
                                  psy[:].rearrange("p (a s) -> p a s", a=2))
            nc.sync.dma_start(
                y_r[:, 2 * otp:2 * otp + 2, SC * j:SC * (j + 1)], ys)

    def emit_outproj(j, evac_act=False):
        for otp in range(D_MODEL // (2 * P)):
            outproj_unit(j, otp, evac_act=evac_act)

    # ---- emission schedule (per-engine FIFO order == execution order) ----
    # group 0 proj interleaved with V proj so PE never waits on the x/wv
    # DMAs (q-scp0 only needs xT chunk 0; vproj st0-7 needs wv + chunks 0-1)
    st_q0 = proj_prepare(0, "q")
    st_k0 = proj_prepare(0, "k")
    qkT = {"q": st_q0["d"], "k": st_k0["d"]}
    # group-0 proj interleaved at half granularity to match DMA arrivals
    proj_half(st_q0, 0, 0)
    proj_half(st_k0, 0, 0)
    proj_half(st_q0, 0, 1)
    proj_half(st_k0, 0, 1)
    proj_half(st_q0, 1, 0)
    proj_half(st_k0, 1, 0)
    proj_half(st_q0, 1, 1)
    proj_half(st_k0, 1, 1)
    emit_vproj(0, 4, evac_act=True)
    proj_rope(st_q0)
    proj_rope(st_k0)
    emit_vproj(4, 12, evac_act=True)

    # group 0: V remainder and group-1 proj woven between attn pairs; units
    # are positioned so every chunk's diagonal tail (tiny PE work per pair)
    # has a PE filler. Rope units emit no PE work -> non-diagonal slots.
    st_q1 = proj_prepare(1, "q")
    st_k1 = proj_prepare(1, "k")
    nxt = {"q": st_q1["d"], "k": st_k1["d"]}
    emit_attn_chunk(0, qkT, 0, fillers=[
        lambda: proj_half(st_q1, 0, 0), lambda: proj_half(st_q1, 0, 1)])
    emit_attn_chunk(0, qkT, 1, fillers=[
        lambda: proj_half(st_q1, 1, 0), lambda: proj_half(st_q1, 1, 1),
        lambda: proj_rope(st_q1), lambda: proj_half(st_k1, 0, 0)])
    emit_attn_chunk(0, qkT, 2, fillers=[
        lambda: proj_half(st_k1, 0, 1), lambda: emit_vproj(12, 13),
        None, None,
        lambda: proj_half(st_k1, 1, 0), lambda: emit_vproj(13, 14)])
    emit_attn_chunk(0, qkT, 3, fillers=[
        lambda: proj_half(st_k1, 1, 1), lambda: proj_rope(st_k1),
        lambda: emit_vproj(14, 15), None, None, None,
        lambda: emit_vproj(15, 16), None])
    qkT = nxt

    # groups 1, 2: next group's proj halves woven between attn pairs
    # (rope finishes inside j3)
    for g in (1, 2):
        st_q = proj_prepare(g + 1, "q")
        st_k = proj_prepare(g + 1, "k")
        nxt = {"q": st_q["d"], "k": st_k["d"]}
        emit_attn_chunk(g, qkT, 0, fillers=[
            lambda: proj_half(st_q, 0, 0), lambda: proj_half(st_q, 0, 1)])
        emit_attn_chunk(g, qkT, 1, fillers=[
            lambda: proj_half(st_q, 1, 0), lambda: proj_half(st_q, 1, 1),
            lambda: proj_rope(st_q), lambda: proj_half(st_k, 0, 0)])
        emit_attn_chunk(g, qkT, 2, fillers=[
            lambda: proj_half(st_k, 0, 1), None, None, None,
            lambda: proj_half(st_k, 1, 0), None])
        emit_attn_chunk(g, qkT, 3, fillers=[
            lambda: proj_half(st_k, 1, 1), lambda: proj_rope(st_k)])
        qkT = nxt

    # group 3: smallest chunk first, then j descending with the previous
    # chunks' out-projections woven in; outproj(1) is the drain tail
    emit_attn_chunk(3, qkT, 0)
    emit_attn_chunk(3, qkT, 3, fillers=[
        None, lambda: outproj_unit(0, 0), lambda: outproj_unit(0, 1),
        None, None, lambda: outproj_unit(0, 2), None,
        lambda: outproj_unit(0, 3)])
    emit_attn_chunk(3, qkT, 2, fillers=[
        lambda: outproj_unit(3, 0), lambda: outproj_unit(3, 1), None, None,
        lambda: outproj_unit(3, 2), lambda: outproj_unit(3, 3)])
    emit_attn_chunk(3, qkT, 1, fillers=[
        lambda: outproj_unit(2, 0), lambda: outproj_unit(2, 1), None,
        lambda: outproj_unit(2, 2), lambda: outproj_unit(2, 3)])
    emit_outproj(1, evac_act=True)


# ---------------------------------------------------------------------------
# Host side
# ---------------------------------------------------------------------------

_NC_CACHE = {}


def _get_nc():
    if "nc" not in _NC_CACHE:
        _NC_CACHE["nc"] = build_nc()
    return _NC_CACHE["nc"]


def _perm64():
    # de-interleave: evens then odds, per 64-dim head
    return np.concatenate([np.arange(0, 64, 2), np.arange(1, 64, 2)])


def make_in_maps(x, token_positions, Wq, Wk, Wv, Wo):
    bf16 = ml_dtypes.bfloat16
    x = np.asarray(x, dtype=np.float32)
    pos = np.asarray(token_positions).astype(np.float32)
    Wq = np.asarray(Wq, dtype=np.float32)
    Wk = np.asarray(Wk, dtype=np.float32)
    Wv = np.asarray(Wv, dtype=np.float32)
    Wo = np.asarray(Wo, dtype=np.float32)

    # RoPE tables in rotate-half (de-interleaved) form, [128, SEQ]:
    # rows 0:32 / 32:64 for head-low/high halves, repeated for partition 64:128
    inv_freq = (10000.0 ** (-np.arange(0, DK, 2, dtype=np.float32)
                            / np.float32(DK))).astype(np.float32)
    ang = inv_freq[:, None] * pos[None, :]            # [32, SEQ]
    cos = np.cos(ang).astype(np.float32)
    sin = np.sin(ang).astype(np.float32)
    cos_t = np.concatenate([cos, cos, cos, cos], axis=0).astype(bf16)
    sin_t = np.concatenate([-sin, sin, -sin, sin], axis=0).astype(bf16)

    # causal diagonal band mask: band[p, c] = 1 if p <= c  (one 128x128 tile)
    pidx = np.arange(P)[:, None]
    cidx = np.arange(P)[None, :]
    mask = np.tile((pidx <= cidx), (1, 2)).astype(bf16)

    perm = _perm64()
    in_maps = []
    for c in range(N_CORES):
        b = c // 2
        hg = c % 2
        rows = slice(HD * hg, HD * (hg + 1))
        # per-head d-permutation for q/k
        qrows = (np.arange(HD).reshape(HPC, DK)[:, perm].reshape(HD)
                 + HD * hg)
        in_maps.append({
            "xT": np.ascontiguousarray(x[b].T).astype(bf16),
            "wqT": np.ascontiguousarray(Wq[qrows, :].T).astype(bf16),
            "wkT": np.ascontiguousarray(Wk[qrows, :].T).astype(bf16),
            "wvT": np.ascontiguousarray(Wv[rows, :].T).astype(bf16),
            "woT": np.ascontiguousarray(Wo[:, rows].T).astype(bf16),
            "cosw": cos_t, "sinw": sin_t, "mask": mask,
        })
    return in_maps


def run(x, token_positions, Wq, Wk, Wv, Wo, trace=False):
    nc = _get_nc()
    in_maps = make_in_maps(x, token_positions, Wq, Wk, Wv, Wo)
    res = run_bass_kernel_spmd(nc, in_maps, list(range(N_CORES)),
                               trace=trace)
    parts = [np.asarray(r["yT"], dtype=np.float32) for r in res.results]
    out = np.stack([(parts[2 * b] + parts[2 * b + 1]).T
                    for b in range(BATCH)]).astype(np.float32)
    return out, res


def kernel(x, token_positions, Wq, Wk, Wv, Wo):
    out, _ = run(x, token_positions, Wq, Wk, Wv, Wo, trace=False)
    return out
